# revision 26
# baseline (speedup 1.0000x reference)
"""Trainium2 Bass kernel for the IWE (image-warped-events) problem, v2.

Full inputs in, full outputs out. Data-parallel over (batch, half) across 8
NeuronCores (core 2b+h gets half h of batch b); host sums the two partial
IWEs per batch.

Per-core pipeline (events [500000,4] padded to [128 x 3968] layout):
  - big contiguous DMAs of raw events (one 0.5 MB transfer per superchunk)
  - per-event flow lookup via GPSIMD ap_gather from a 128-partition
    replicated bf16 (fy,fx) table indexed by y*128+x
  - bilinear "hat" weight rows built per 128-event block:
    m = |iota - warped| (DVE stt), hatY = min(m,1)-1 (DVE ts, negated),
    hatX = relu(1-m) (ACT)
  - polarity folded into the x grid: wx += 146*p -> one 288-wide scatter
    matmul per block accumulates PSUM [128,288]: neg grid cols 0:128,
    pos grid cols 146:274 (gutters absorb out-of-bounds corners, matching
    the reference's OOB masking; pad events use p=3 -> fully off-grid)
"""
import numpy as np
import ml_dtypes

H, W = 128, 128
NCORES = 8
CHUNK = 500                    # kept for test.py's cache-key computation
E_REAL = 500000                # events per core (N/2)
NCOLS = 3968
E_PAD = 128 * NCOLS            # 507904
NSC = 16                       # superchunks
SC = NCOLS // NSC              # 248 event-columns per superchunk
NI = 16 * SC                   # gather idxs per Q7 core per superchunk
W2 = 288                       # packed x-grid width (neg | gutter | pos | gutter)
OFF = 146                      # positive-polarity column offset
NB = 8                         # blocks per hat group

_COMPILED = {}


def _build(nchunks, use_hw_loop=True, unroll=2, passes=1):
    import concourse.bass as bass
    import concourse.bacc as bacc
    import concourse.mybir as mybir
    from concourse.tile import TileContext

    fp32 = mybir.dt.float32
    bf16 = mybir.dt.bfloat16
    int16 = mybir.dt.int16
    int32 = mybir.dt.int32
    Alu = mybir.AluOpType
    Act = mybir.ActivationFunctionType

    nc = bacc.Bacc("TRN2", target_bir_lowering=False, debug=False,
                   num_devices=NCORES)

    ev = nc.dram_tensor("ev", [E_PAD, 4], fp32, kind="ExternalInput").ap()
    ftab = nc.dram_tensor("ftab", [2 * H * W], bf16, kind="ExternalInput").ap()
    flow = nc.dram_tensor("flow", [2, H, W], fp32, kind="ExternalInput").ap()
    emask = nc.dram_tensor("emask", [H, W], fp32, kind="ExternalInput").ap()
    selin = nc.dram_tensor("sel16", [128, 32], bf16, kind="ExternalInput").ap()
    out = nc.dram_tensor("out", [4, H, W], fp32, kind="ExternalOutput").ap()

    ev_v = ev.rearrange("(p s c) f -> p s (c f)", p=128, s=NSC, c=SC)

    with TileContext(nc) as tc:
        with tc.tile_pool(name="const", bufs=1) as cpool, \
             tc.tile_pool(name="work", bufs=2) as wpool, \
             tc.tile_pool(name="hats", bufs=2) as hpool, \
             tc.tile_pool(name="ppool", bufs=1, space="PSUM") as ppool:

            # ---------------- constants ----------------
            iotai = cpool.tile([128, W2], int32)
            nc.gpsimd.iota(iotai[:], pattern=[[1, W2]], base=0,
                           channel_multiplier=0)
            iota = cpool.tile([128, W2], bf16)
            nc.vector.tensor_copy(out=iota[:], in_=iotai[:])
            # centered x-basis: values -144..143, all exactly representable
            # in bf16 (0..287 is not: integers >256 round to even)
            iotaf = cpool.tile([128, W2], fp32)
            nc.vector.tensor_copy(out=iotaf[:], in_=iotai[:])
            iotac = cpool.tile([128, W2], bf16)
            nc.vector.tensor_scalar(out=iotac[:], in0=iotaf[:],
                                    scalar1=-float(W2 // 2), scalar2=None,
                                    op0=Alu.add)
            zero = cpool.tile([128, W2], bf16)
            nc.vector.memset(zero[:], 0.0)
            c146 = cpool.tile([128, 1], fp32)
            nc.vector.memset(c146[:], float(OFF))
            cm144 = cpool.tile([128, 1], fp32)
            nc.vector.memset(cm144[:], -float(W2 // 2))
            c128 = cpool.tile([128, 1], fp32)
            nc.vector.memset(c128[:], 128.0)
            cinv = cpool.tile([128, 1], fp32)
            nc.vector.memset(cinv[:], 1.0 / (1.0 + 1e-9))

            # sel16[q, 2k+t] = (k == q%16): per-partition slot mask used to
            # extract each event's value from the core-replicated gather out
            sel16 = cpool.tile([128, 32], bf16)
            nc.sync.dma_start(out=sel16[:], in_=selin)

            table = cpool.tile([128, 2 * H * W], bf16)
            nc.sync.dma_start(
                out=table[:],
                in_=ftab.unsqueeze(0).broadcast_to([128, 2 * H * W]))

            flow32 = cpool.tile([128, 256], fp32)
            nc.sync.dma_start(out=flow32[:, 0:128], in_=flow[0])
            nc.sync.dma_start(out=flow32[:, 128:256], in_=flow[1])
            maskt = cpool.tile([128, 128], fp32)
            nc.sync.dma_start(out=maskt[:], in_=emask[:, :])

            acc = ppool.tile([128, W2], fp32, tag="acc")
            nc.vector.memset(acc[:], 0.0)



            # ---------------- event pipeline ----------------
            def body(s):
                evt = wpool.tile([128, SC * 4], fp32, tag="evt")
                nc.sync.dma_start(out=evt[:], in_=ev_v[:, bass.ds(s, 1), :])
                ev3 = evt[:].rearrange("p (c f) -> p c f", f=4)
                tsv = ev3[:, :, 0]
                yv = ev3[:, :, 1]
                xv = ev3[:, :, 2]
                pv = ev3[:, :, 3]

                idxf = wpool.tile([128, SC], fp32, tag="idxf")
                nc.vector.scalar_tensor_tensor(
                    out=idxf[:], in0=yv, scalar=c128[:], in1=xv,
                    op0=Alu.mult, op1=Alu.add)
                idxi = wpool.tile([128, SC], int16, tag="idxi")
                nc.vector.tensor_copy(out=idxi[:], in_=idxf[:])

                gout = wpool.tile([128, NI * 2], bf16, tag="gout")
                nc.gpsimd.ap_gather(
                    gout[:], table[:], idxi[:],
                    channels=128, num_elems=H * W, d=2, num_idxs=NI)

                # each partition keeps only its own slot (k == q%16) of the
                # 16-replicated gather output, then sums over the 16 slots
                gmsk = wpool.tile([128, NI * 2], bf16, tag="gmsk")
                nc.vector.tensor_tensor(
                    out=gmsk[:].rearrange("p (j kt) -> p j kt", kt=32),
                    in0=gout[:].rearrange("p (j kt) -> p j kt", kt=32),
                    in1=sel16[:].unsqueeze(1).broadcast_to([128, SC, 32]),
                    op=Alu.mult)
                fyfx = wpool.tile([128, SC * 2], fp32, tag="fyfx")
                f3 = fyfx[:].rearrange("p (j two) -> p j two", two=2)
                nc.vector.tensor_reduce(
                    out=f3,
                    in_=gmsk[:].rearrange("p (j k two) -> p j two k",
                                          k=16, two=2),
                    axis=mybir.AxisListType.X, op=Alu.add)
                fy = f3[:, :, 0]
                fx = f3[:, :, 1]

                u = wpool.tile([128, SC], fp32, tag="u")
                nc.vector.tensor_scalar(out=u[:], in0=tsv, scalar1=-1.0,
                                        scalar2=1.0, op0=Alu.mult, op1=Alu.add)
                t1 = wpool.tile([128, SC], fp32, tag="t1")
                nc.vector.tensor_tensor(out=t1[:], in0=u[:], in1=fy, op=Alu.mult)
                wy = wpool.tile([128, SC], fp32, tag="wy")
                nc.vector.tensor_tensor(out=wy[:], in0=t1[:], in1=yv, op=Alu.add)
                t2 = wpool.tile([128, SC], fp32, tag="t2")
                nc.vector.tensor_tensor(out=t2[:], in0=u[:], in1=fx, op=Alu.mult)
                t3 = wpool.tile([128, SC], fp32, tag="t3")
                nc.vector.scalar_tensor_tensor(
                    out=t3[:], in0=xv, scalar=cm144[:], in1=t2[:],
                    op0=Alu.add, op1=Alu.add)
                wx = wpool.tile([128, SC], fp32, tag="wx")
                nc.vector.scalar_tensor_tensor(
                    out=wx[:], in0=pv, scalar=c146[:], in1=t3[:],
                    op0=Alu.mult, op1=Alu.add)

                for g in range(SC // NB):
                    TY = hpool.tile([128, NB * 128], bf16, tag="TY")
                    TNY = hpool.tile([128, NB * 128], bf16, tag="TNY")
                    MY = hpool.tile([128, NB * 128], bf16, tag="MY")
                    HY = hpool.tile([128, NB * 128], bf16, tag="HY")
                    TX = hpool.tile([128, NB * W2], bf16, tag="TX")
                    AX = hpool.tile([128, NB * W2], bf16, tag="AX")
                    HX = hpool.tile([128, NB * W2], bf16, tag="HX")
                    for b in range(NB):
                        j = g * NB + b
                        nc.vector.scalar_tensor_tensor(
                            out=TY[:, b * 128:(b + 1) * 128],
                            in0=iota[:, 0:128], scalar=wy[:, j:j + 1],
                            in1=zero[:, 0:128],
                            op0=Alu.subtract, op1=Alu.add)
                        nc.vector.scalar_tensor_tensor(
                            out=TX[:, b * W2:(b + 1) * W2],
                            in0=iotac[:], scalar=wx[:, j:j + 1], in1=zero[:],
                            op0=Alu.subtract, op1=Alu.add)
                    # negated hatY: min(|t|,1)-1 in [-1,0]
                    nc.vector.tensor_scalar_mul(out=TNY[:], in0=TY[:],
                                                scalar1=-1.0)
                    nc.vector.tensor_tensor(out=MY[:], in0=TY[:], in1=TNY[:],
                                            op=Alu.max)
                    nc.vector.tensor_scalar(out=HY[:], in0=MY[:], scalar1=1.0,
                                            scalar2=1.0, op0=Alu.min,
                                            op1=Alu.subtract)
                    # positive hatX: relu(1-|t|) via ACT
                    nc.scalar.activation(out=AX[:], in_=TX[:], func=Act.Abs)
                    nc.scalar.activation(out=HX[:], in_=AX[:], func=Act.Relu,
                                         bias=1.0, scale=-1.0)
                    for b in range(NB):
                        nc.tensor.matmul(
                            out=acc[:],
                            lhsT=HY[:, b * 128:(b + 1) * 128],
                            rhs=HX[:, b * W2:(b + 1) * W2],
                            start=False, stop=False)

            # event pipeline; passes>1 (timing variant) wraps the superchunk
            # loop in a static outer HW loop to amplify the measured work
            if passes == 1:
                tc.For_i_unrolled(0, NSC, 1, body, max_unroll=unroll)
            else:
                with tc.For_i(0, passes):
                    tc.For_i_unrolled(0, NSC, 1, body, max_unroll=unroll)

            # ---------------- finalize ----------------
            accsb = cpool.tile([128, W2], fp32)
            nc.vector.tensor_copy(out=accsb[:], in_=acc[:])
            res = cpool.tile([128, 512], fp32)
            # acc holds -(true grids): negate back
            nc.vector.tensor_scalar_mul(out=res[:, 0:128],
                                        in0=accsb[:, OFF:OFF + 128],
                                        scalar1=-1.0)
            nc.vector.tensor_scalar_mul(out=res[:, 128:256],
                                        in0=accsb[:, 0:128], scalar1=-1.0)
            nc.vector.scalar_tensor_tensor(
                out=res[:, 256:384], in0=flow32[:, 0:128], scalar=cinv[:],
                in1=maskt[:], op0=Alu.mult, op1=Alu.mult)
            nc.vector.scalar_tensor_tensor(
                out=res[:, 384:512], in0=flow32[:, 128:256], scalar=cinv[:],
                in1=maskt[:], op0=Alu.mult, op1=Alu.mult)
            for ch in range(4):
                nc.sync.dma_start(out=out[ch],
                                  in_=res[:, ch * 128:(ch + 1) * 128])

    nc.compile()
    return nc


def _run(nc, flow, event_list, pol_mask, event_mask):
    """flow [B,2,H,W], event_list [B,N,4], pol [B,N,2], emask [B,1,H,W]."""
    from concourse.bass_utils import run_bass_kernel_spmd

    Bb, Nn = event_list.shape[0], event_list.shape[1]
    half = Nn // 2
    pad = np.zeros((E_PAD - E_REAL, 4), np.float32)
    pad[:, 3] = 3.0              # p=3 -> x-offset 438: fully off-grid
    sel16 = np.zeros((128, 32), ml_dtypes.bfloat16)
    for q in range(128):
        sel16[q, 2 * (q % 16)] = 1.0
        sel16[q, 2 * (q % 16) + 1] = 1.0
    ftabs = []
    for b in range(Bb):
        t = np.empty(2 * H * W, ml_dtypes.bfloat16)
        t[0::2] = flow[b, 1].ravel().astype(ml_dtypes.bfloat16)   # fy
        t[1::2] = flow[b, 0].ravel().astype(ml_dtypes.bfloat16)   # fx
        ftabs.append(t)
    in_maps = []
    for c in range(NCORES):
        b, h = c // 2, c % 2
        sl = slice(h * half, (h + 1) * half)
        evc = np.concatenate(
            [np.ascontiguousarray(event_list[b, sl, :], np.float32), pad])
        in_maps.append({
            "ev": evc,
            "ftab": ftabs[b],
            "flow": np.ascontiguousarray(flow[b], np.float32),
            "emask": np.ascontiguousarray(event_mask[b, 0], np.float32),
            "sel16": sel16,
        })
    res = run_bass_kernel_spmd(nc, in_maps, list(range(NCORES)))
    outp = np.zeros((Bb, 4, H, W), np.float32)
    for c in range(NCORES):
        b = c // 2
        r = res.results[c]["out"]
        outp[b, 0:2] += r[0:2]
        if c % 2 == 0:
            outp[b, 2:4] = r[2:4]
    return outp


def kernel(flow, event_list, pol_mask, event_mask):
    flow = np.asarray(flow, np.float32)
    event_list = np.asarray(event_list, np.float32)
    pol_mask = np.asarray(pol_mask, np.float32)
    event_mask = np.asarray(event_mask, np.float32)
    nchunks = event_list.shape[0] * event_list.shape[1] // NCORES // CHUNK
    key = ("nc", nchunks)
    if key not in _COMPILED:
        _COMPILED[key] = _build(nchunks)
    return _run(_COMPILED[key], flow, event_list, pol_mask, event_mask)


# revision 32
# speedup vs baseline: 1.3155x; 1.3155x over previous
"""Trainium2 Bass kernel for the IWE (image-warped-events) problem, v2.

Full inputs in, full outputs out. Data-parallel over (batch, half) across 8
NeuronCores (core 2b+h gets half h of batch b); host sums the two partial
IWEs per batch.

Per-core pipeline (events [500000,4] padded to [128 x 3968] layout):
  - big contiguous DMAs of raw events (one 0.5 MB transfer per superchunk)
  - per-event flow lookup via GPSIMD ap_gather from a 128-partition
    replicated bf16 (fy,fx) table indexed by y*128+x
  - bilinear "hat" weight rows built per 128-event block:
    m = |iota - warped| (DVE stt), hatY = min(m,1)-1 (DVE ts, negated),
    hatX = relu(1-m) (ACT)
  - polarity folded into the x grid: wx += 146*p -> one 288-wide scatter
    matmul per block accumulates PSUM [128,288]: neg grid cols 0:128,
    pos grid cols 146:274 (gutters absorb out-of-bounds corners, matching
    the reference's OOB masking; pad events use p=3 -> fully off-grid)
"""
import numpy as np
import ml_dtypes

H, W = 128, 128
NCORES = 8
CHUNK = 500                    # kept for test.py's cache-key computation
E_REAL = 500000                # events per core (N/2)
NCOLS = 3968
E_PAD = 128 * NCOLS            # 507904
NSC = 16                       # superchunks
SC = NCOLS // NSC              # 248 event-columns per superchunk
NI = 16 * SC                   # gather idxs per Q7 core per superchunk
W2 = 288                       # packed x-grid width (neg | gutter | pos | gutter)
OFF = 146                      # positive-polarity column offset
NB = 8                         # blocks per hat group

_COMPILED = {}


def _build(nchunks, use_hw_loop=True, unroll=2, passes=1, ablate=None):
    import concourse.bass as bass
    import concourse.bacc as bacc
    import concourse.mybir as mybir
    from concourse.tile import TileContext

    fp32 = mybir.dt.float32
    bf16 = mybir.dt.bfloat16
    int16 = mybir.dt.int16
    int32 = mybir.dt.int32
    Alu = mybir.AluOpType
    Act = mybir.ActivationFunctionType

    nc = bacc.Bacc("TRN2", target_bir_lowering=False, debug=False,
                   num_devices=NCORES)

    ev = nc.dram_tensor("ev", [E_PAD, 4], fp32, kind="ExternalInput").ap()
    ftab = nc.dram_tensor("ftab", [2 * H * W], bf16, kind="ExternalInput").ap()
    flow = nc.dram_tensor("flow", [2, H, W], fp32, kind="ExternalInput").ap()
    emask = nc.dram_tensor("emask", [H, W], fp32, kind="ExternalInput").ap()
    selin = nc.dram_tensor("sel16", [128, 32], bf16, kind="ExternalInput").ap()
    out = nc.dram_tensor("out", [4, H, W], fp32, kind="ExternalOutput").ap()

    ev_v = ev.rearrange("(p s c) f -> p s (c f)", p=128, s=NSC, c=SC)

    with TileContext(nc) as tc:
        with tc.tile_pool(name="const", bufs=1) as cpool, \
             tc.tile_pool(name="work", bufs=2) as wpool, \
             tc.tile_pool(name="hats", bufs=2) as hpool, \
             tc.tile_pool(name="ppool", bufs=1, space="PSUM") as ppool:

            # ---------------- constants ----------------
            iotai = cpool.tile([128, W2], int32)
            nc.gpsimd.iota(iotai[:], pattern=[[1, W2]], base=0,
                           channel_multiplier=0)
            iota = cpool.tile([128, W2], bf16)
            nc.vector.tensor_copy(out=iota[:], in_=iotai[:])
            # centered x-basis: values -144..143, all exactly representable
            # in bf16 (0..287 is not: integers >256 round to even)
            iotaf = cpool.tile([128, W2], fp32)
            nc.vector.tensor_copy(out=iotaf[:], in_=iotai[:])
            iotac = cpool.tile([128, W2], bf16)
            nc.vector.tensor_scalar(out=iotac[:], in0=iotaf[:],
                                    scalar1=-float(W2 // 2), scalar2=None,
                                    op0=Alu.add)
            zero = cpool.tile([128, W2], bf16)
            nc.vector.memset(zero[:], 0.0)
            c146 = cpool.tile([128, 1], fp32)
            nc.vector.memset(c146[:], float(OFF))
            cm144 = cpool.tile([128, 1], fp32)
            nc.vector.memset(cm144[:], -float(W2 // 2))
            c128 = cpool.tile([128, 1], fp32)
            nc.vector.memset(c128[:], 128.0)
            cinv = cpool.tile([128, 1], fp32)
            nc.vector.memset(cinv[:], 1.0 / (1.0 + 1e-9))

            # sel16[q, 2k+t] = (k == q%16): per-partition slot mask used to
            # extract each event's value from the core-replicated gather out
            sel16 = cpool.tile([128, 32], bf16)
            nc.sync.dma_start(out=sel16[:], in_=selin)

            table = cpool.tile([128, 2 * H * W], bf16)
            nc.sync.dma_start(
                out=table[:],
                in_=ftab.unsqueeze(0).broadcast_to([128, 2 * H * W]))

            flow32 = cpool.tile([128, 256], fp32)
            nc.sync.dma_start(out=flow32[:, 0:128], in_=flow[0])
            nc.sync.dma_start(out=flow32[:, 128:256], in_=flow[1])
            maskt = cpool.tile([128, 128], fp32)
            nc.sync.dma_start(out=maskt[:], in_=emask[:, :])

            acc = ppool.tile([128, W2], fp32, tag="acc")
            nc.vector.memset(acc[:], 0.0)



            # ---------------- event pipeline ----------------
            def body(s):
                evt = wpool.tile([128, SC * 4], fp32, tag="evt")
                nc.sync.dma_start(out=evt[:], in_=ev_v[:, bass.ds(s, 1), :])
                ev3 = evt[:].rearrange("p (c f) -> p c f", f=4)
                tsv = ev3[:, :, 0]
                yv = ev3[:, :, 1]
                xv = ev3[:, :, 2]
                pv = ev3[:, :, 3]

                idxf = wpool.tile([128, SC], fp32, tag="idxf")
                nc.vector.scalar_tensor_tensor(
                    out=idxf[:], in0=yv, scalar=c128[:], in1=xv,
                    op0=Alu.mult, op1=Alu.add)
                idxi = wpool.tile([128, SC], int16, tag="idxi")
                nc.vector.tensor_copy(out=idxi[:], in_=idxf[:])

                fyfx = wpool.tile([128, SC * 2], fp32, tag="fyfx")
                f3 = fyfx[:].rearrange("p (j two) -> p j two", two=2)
                if ablate != "nogather":
                    gout = wpool.tile([128, NI * 2], bf16, tag="gout")
                    nc.gpsimd.ap_gather(
                        gout[:], table[:], idxi[:],
                        channels=128, num_elems=H * W, d=2, num_idxs=NI)

                    # each partition keeps only its own slot (k == q%16) of
                    # the 16-replicated gather output, then sums the 16 slots
                    gmsk = wpool.tile([128, NI * 2], bf16, tag="gmsk")
                    nc.vector.tensor_tensor(
                        out=gmsk[:].rearrange("p (j kt) -> p j kt", kt=32),
                        in0=gout[:].rearrange("p (j kt) -> p j kt", kt=32),
                        in1=sel16[:].unsqueeze(1).broadcast_to([128, SC, 32]),
                        op=Alu.mult)
                    nc.vector.tensor_reduce(
                        out=f3,
                        in_=gmsk[:].rearrange("p (j k two) -> p j two k",
                                              k=16, two=2),
                        axis=mybir.AxisListType.X, op=Alu.add)
                else:
                    nc.vector.memset(fyfx[:], 0.25)
                fy = f3[:, :, 0]
                fx = f3[:, :, 1]

                u = wpool.tile([128, SC], fp32, tag="u")
                nc.vector.tensor_scalar(out=u[:], in0=tsv, scalar1=-1.0,
                                        scalar2=1.0, op0=Alu.mult, op1=Alu.add)
                t1 = wpool.tile([128, SC], fp32, tag="t1")
                nc.vector.tensor_tensor(out=t1[:], in0=u[:], in1=fy, op=Alu.mult)
                wy = wpool.tile([128, SC], fp32, tag="wy")
                nc.vector.tensor_tensor(out=wy[:], in0=t1[:], in1=yv, op=Alu.add)
                t2 = wpool.tile([128, SC], fp32, tag="t2")
                nc.vector.tensor_tensor(out=t2[:], in0=u[:], in1=fx, op=Alu.mult)
                t3 = wpool.tile([128, SC], fp32, tag="t3")
                nc.vector.scalar_tensor_tensor(
                    out=t3[:], in0=xv, scalar=cm144[:], in1=t2[:],
                    op0=Alu.add, op1=Alu.add)
                wx = wpool.tile([128, SC], fp32, tag="wx")
                nc.vector.scalar_tensor_tensor(
                    out=wx[:], in0=pv, scalar=c146[:], in1=t3[:],
                    op0=Alu.mult, op1=Alu.add)

                if ablate == "nohats":
                    return
                for g in range(SC // NB):
                    TY = hpool.tile([128, NB * 128], bf16, tag="TY")
                    TNY = hpool.tile([128, NB * 128], bf16, tag="TNY")
                    MY = hpool.tile([128, NB * 128], bf16, tag="MY")
                    HY = hpool.tile([128, NB * 128], bf16, tag="HY")
                    TX = hpool.tile([128, NB * W2], bf16, tag="TX")
                    AX = hpool.tile([128, NB * W2], bf16, tag="AX")
                    HX = hpool.tile([128, NB * W2], bf16, tag="HX")
                    gs = slice(g * NB, (g + 1) * NB)
                    nc.vector.tensor_tensor(
                        out=TY[:].rearrange("p (b f) -> p b f", f=128),
                        in0=iota[:, 0:128].unsqueeze(1).broadcast_to(
                            [128, NB, 128]),
                        in1=wy[:, gs].unsqueeze(2).broadcast_to(
                            [128, NB, 128]),
                        op=Alu.subtract)
                    nc.vector.tensor_tensor(
                        out=TX[:].rearrange("p (b f) -> p b f", f=W2),
                        in0=iotac[:].unsqueeze(1).broadcast_to(
                            [128, NB, W2]),
                        in1=wx[:, gs].unsqueeze(2).broadcast_to(
                            [128, NB, W2]),
                        op=Alu.subtract)
                    # negated hatY: min(|t|,1)-1 in [-1,0]
                    nc.vector.tensor_scalar_mul(out=TNY[:], in0=TY[:],
                                                scalar1=-1.0)
                    nc.vector.tensor_tensor(out=MY[:], in0=TY[:], in1=TNY[:],
                                            op=Alu.max)
                    nc.vector.tensor_scalar(out=HY[:], in0=MY[:], scalar1=1.0,
                                            scalar2=1.0, op0=Alu.min,
                                            op1=Alu.subtract)
                    # positive hatX: relu(1-|t|) via ACT
                    nc.scalar.activation(out=AX[:], in_=TX[:], func=Act.Abs)
                    nc.scalar.activation(out=HX[:], in_=AX[:], func=Act.Relu,
                                         bias=1.0, scale=-1.0)
                    if ablate == "nomm":
                        continue
                    for b in range(NB):
                        nc.tensor.matmul(
                            out=acc[:],
                            lhsT=HY[:, b * 128:(b + 1) * 128],
                            rhs=HX[:, b * W2:(b + 1) * W2],
                            start=False, stop=False)

            # event pipeline, fully unrolled over superchunks (no inner HW
            # loop: its per-back-edge all-engine barrier would serialize the
            # GPSIMD gather against the DVE/ACT hat work); passes>1 (timing
            # variant) wraps it in a static outer HW loop
            def pipeline():
                for s in range(NSC):
                    body(s)

            if passes == 1:
                pipeline()
            else:
                with tc.For_i(0, passes):
                    pipeline()

            # ---------------- finalize ----------------
            accsb = cpool.tile([128, W2], fp32)
            nc.vector.tensor_copy(out=accsb[:], in_=acc[:])
            res = cpool.tile([128, 512], fp32)
            # acc holds -(true grids): negate back
            nc.vector.tensor_scalar_mul(out=res[:, 0:128],
                                        in0=accsb[:, OFF:OFF + 128],
                                        scalar1=-1.0)
            nc.vector.tensor_scalar_mul(out=res[:, 128:256],
                                        in0=accsb[:, 0:128], scalar1=-1.0)
            nc.vector.scalar_tensor_tensor(
                out=res[:, 256:384], in0=flow32[:, 0:128], scalar=cinv[:],
                in1=maskt[:], op0=Alu.mult, op1=Alu.mult)
            nc.vector.scalar_tensor_tensor(
                out=res[:, 384:512], in0=flow32[:, 128:256], scalar=cinv[:],
                in1=maskt[:], op0=Alu.mult, op1=Alu.mult)
            for ch in range(4):
                nc.sync.dma_start(out=out[ch],
                                  in_=res[:, ch * 128:(ch + 1) * 128])

    nc.compile()
    return nc


def _run(nc, flow, event_list, pol_mask, event_mask):
    """flow [B,2,H,W], event_list [B,N,4], pol [B,N,2], emask [B,1,H,W]."""
    from concourse.bass_utils import run_bass_kernel_spmd

    Bb, Nn = event_list.shape[0], event_list.shape[1]
    half = Nn // 2
    pad = np.zeros((E_PAD - E_REAL, 4), np.float32)
    pad[:, 3] = 3.0              # p=3 -> x-offset 438: fully off-grid
    sel16 = np.zeros((128, 32), ml_dtypes.bfloat16)
    for q in range(128):
        sel16[q, 2 * (q % 16)] = 1.0
        sel16[q, 2 * (q % 16) + 1] = 1.0
    ftabs = []
    for b in range(Bb):
        t = np.empty(2 * H * W, ml_dtypes.bfloat16)
        t[0::2] = flow[b, 1].ravel().astype(ml_dtypes.bfloat16)   # fy
        t[1::2] = flow[b, 0].ravel().astype(ml_dtypes.bfloat16)   # fx
        ftabs.append(t)
    in_maps = []
    for c in range(NCORES):
        b, h = c // 2, c % 2
        sl = slice(h * half, (h + 1) * half)
        evc = np.concatenate(
            [np.ascontiguousarray(event_list[b, sl, :], np.float32), pad])
        in_maps.append({
            "ev": evc,
            "ftab": ftabs[b],
            "flow": np.ascontiguousarray(flow[b], np.float32),
            "emask": np.ascontiguousarray(event_mask[b, 0], np.float32),
            "sel16": sel16,
        })
    res = run_bass_kernel_spmd(nc, in_maps, list(range(NCORES)))
    outp = np.zeros((Bb, 4, H, W), np.float32)
    for c in range(NCORES):
        b = c // 2
        r = res.results[c]["out"]
        outp[b, 0:2] += r[0:2]
        if c % 2 == 0:
            outp[b, 2:4] = r[2:4]
    return outp


def kernel(flow, event_list, pol_mask, event_mask):
    flow = np.asarray(flow, np.float32)
    event_list = np.asarray(event_list, np.float32)
    pol_mask = np.asarray(pol_mask, np.float32)
    event_mask = np.asarray(event_mask, np.float32)
    nchunks = event_list.shape[0] * event_list.shape[1] // NCORES // CHUNK
    key = ("nc", nchunks)
    if key not in _COMPILED:
        _COMPILED[key] = _build(nchunks)
    return _run(_COMPILED[key], flow, event_list, pol_mask, event_mask)
